# revision 28
# baseline (speedup 1.0000x reference)
"""Trainium2 Bass kernel for nn_CapacitanceMatrix.

C[b, i, j] = sigmoid(x[b]·Wd[i] + bd[i])        if i == j
           = -softplus(x[b]·Wo[m] + bo[m])      if i != j  (m = row-major off-diag idx)

Output-stationary layout: the 256 output rows (permuted: 16 diag rows first)
sit on PSUM partitions in 2 halves of 128; batch streams as the moving
operand in 512-col blocks, 8 PSUM banks <- (half g, block jb), accumulating
the 8 D-chunks. Every matmul pays its own stationary load (measured: no
elision on TRN2), so instruction order is chosen purely for pipelining:

- supergroups 0..2 run chunk-outer (c, g, jb) so the PE trails the streaming
  x DMA chunk by chunk;
- the last supergroup runs bank-major (g, jb, c) — its x is fully prefetched
  by then — so banks complete evenly through the group and the Exp/Ln/DMA
  drain tail shrinks from ~13us to ~4us.

Device computes v = softplus(W'x + b') uniformly for all rows, with diag rows
hosting W' = -Wd, b' = -bd so that v_diag = softplus(-z_d) = -ln(sigmoid(z_d)).
Host finishes: off-diag C = -v, diag C = exp(-v) = sigmoid(z_d). The bias
rides the Exp activation's per-partition bias AP (no PSUM seeding, no vector
engine). Output is fp16 (host upcasts), halving out-traffic: 21.5MB/core.
"""

import os
import sys

sys.path.insert(0, "/opt/trn_rl_repo")

from contextlib import ExitStack

import numpy as np

import concourse.bass as bass  # noqa: F401  (AP helpers)
import concourse.tile as tile
from concourse import bacc, mybir
from concourse.bass_utils import run_bass_kernel_spmd

B = 65536
D = 1024
K = 16
NOUT = K * K  # 256
NCORES = 8
BC = B // NCORES  # 8192 batch rows per core
KD = D // 128  # 8 contraction chunks
SGC = 2048  # supergroup width
NSG = BC // SGC  # 4
JBLK = 512  # cols per matmul / psum bank
NJB = SGC // JBLK  # 4

# matmul dtype for x / weights ("bfloat16" default)
MM_DT_NAME = os.environ.get("CAP_MM_DT", "bfloat16")

_CACHE = {}

_ACT_TABLES_PATCHED = False


def _pin_act_table_set():
    """Force Exp and Ln to resolve to the single LUT set that holds both
    (`natural_log_exp_and_others`) so the Exp/Ln alternation doesn't thrash
    ACT_TABLE_LOADs."""
    global _ACT_TABLES_PATCHED
    if _ACT_TABLES_PATCHED:
        return
    import concourse.hw_specs as hw_specs

    orig = hw_specs.get_activation_tables

    def patched(arch):
        tables = {k: set(v) for k, v in orig(arch).items()}
        keep = "natural_log_exp_and_others"
        if keep in tables:
            for k, v in tables.items():
                if k != keep:
                    v.discard(mybir.ActivationFunctionType.Exp)
                    v.discard(mybir.ActivationFunctionType.Ln)
        return tables

    bacc.get_activation_tables = patched
    _ACT_TABLES_PATCHED = True


def _mm_dt():
    return getattr(mybir.dt, MM_DT_NAME)


def _np_dt():
    return mybir.dt.np(_mm_dt())


def _perm():
    """Device row r -> original flat output index (i*16+j)."""
    off_i, off_j = np.nonzero(~np.eye(K, dtype=bool))
    perm = np.empty(NOUT, np.int64)
    perm[:K] = np.arange(K) * (K + 1)
    perm[K:] = off_i * K + off_j
    return perm


def _build_bass():
    _pin_act_table_set()
    mm_dt = _mm_dt()
    f32 = mybir.dt.float32
    f16 = mybir.dt.float16
    nc = bacc.Bacc("TRN2", target_bir_lowering=False, debug=False)
    # x pre-tiled on host: [chunk, partition(d), col(batch)] per core
    xT = nc.dram_tensor("xT", [KD, 128, BC], mm_dt, kind="ExternalInput").ap()
    # wt[p, c, g, n] = W'^T[c*128+p, g*128+n]
    wt = nc.dram_tensor("wt", [128, KD, 2, 128], mm_dt, kind="ExternalInput").ap()
    # bvec[p, g] = b'[g*128+p]
    bvec = nc.dram_tensor("bvec", [128, 2], f32, kind="ExternalInput").ap()
    # out[g, r, col] = softplus value for device row g*128+r, batch col
    out = nc.dram_tensor("out", [2, 128, BC], f16, kind="ExternalOutput").ap()

    EXP = mybir.ActivationFunctionType.Exp
    LN = mybir.ActivationFunctionType.Ln

    with tile.TileContext(nc) as tc, ExitStack() as ctx:
        const_pool = ctx.enter_context(tc.tile_pool(name="const", bufs=1))
        x_pool = ctx.enter_context(tc.tile_pool(name="x", bufs=3 * KD))
        ev_pool = ctx.enter_context(tc.tile_pool(name="ev", bufs=6))
        ot_pool = ctx.enter_context(tc.tile_pool(name="ot", bufs=4))
        psum_pool = ctx.enter_context(tc.tile_pool(name="ps", bufs=8, space="PSUM"))

        wt_sb = const_pool.tile([128, KD, 2, 128], mm_dt)
        bv_sb = const_pool.tile([128, 2], f32)
        # chunk-0 weights ride sync so the very first trigger serves the
        # first matmul; the rest trickle in on scalar (needed much later)
        nc.sync.dma_start(wt_sb[:, 0], wt[:, 0])
        nc.scalar.dma_start(bv_sb[:], bvec)
        for c in range(1, KD):
            nc.scalar.dma_start(wt_sb[:, c], wt[:, c])

        dma_engines = [nc.sync, nc.gpsimd]
        # All tiles are allocated once and rotated manually: the end-of-kernel
        # barrier pays ~115ns per logical tile per engine (~57 EVENT_SEMAPHORE
        # slices/engine with per-supergroup allocation), so fewer logical
        # tiles directly shrink the ~10us drain storm.
        # 2 supergroups of x in flight: with 3, sg2's prefetch DMAs compete
        # with sg0's just-in-time chunks for HBM and starve the PE early on
        NXT = 2 * KD
        x_t = [x_pool.tile([128, SGC], mm_dt, name="x") for _ in range(NXT)]
        ps_t = [psum_pool.tile([128, JBLK], f32, name="ps") for _ in range(2 * NJB)]
        ev_t = [ev_pool.tile([128, 2, JBLK], f32, name="ev") for _ in range(NJB)]
        ot_t = [ot_pool.tile([128, SGC], f16, name="ot") for _ in range(4)]

        pending_outs = []  # out DMAs deferred one supergroup (waits satisfied)
        for sg in range(NSG):
            col0 = sg * SGC
            x_sb = []
            for c in range(KD):
                xc = x_t[(sg * KD + c) % NXT]
                # chunk 0 of sg 0 lands fine-grained so the first matmul can
                # start within ~2us; everything else uses 2KB-run halves.
                # Pieces alternate sync/gpsimd so one sequencer's issue rate
                # never serializes a chunk's arrival.
                npc = 4 if (sg == 0 and c == 0) else 2
                w = SGC // npc
                for q in range(npc):
                    # sg0's last chunks ride scalar (idle until the first
                    # Exp at ~17us) so the 600ns-per-trigger issue rate on
                    # sync/gpsimd doesn't throttle early queue ramp-up
                    if sg == 0 and c >= 6:
                        eng = nc.scalar
                    else:
                        eng = dma_engines[(c * npc + q) % 2]
                    eng.dma_start(
                        xc[:, q * w : (q + 1) * w],
                        xT[c, :, col0 + q * w : col0 + (q + 1) * w],
                    )
                x_sb.append(xc)
                # trickle out one deferred out-DMA per odd chunk: spreads the
                # 1MB of out traffic through the supergroup instead of
                # bunching it against the first chunks' arrival
                if c % 2 == 1 and pending_outs:
                    dst, src = pending_outs.pop(0)
                    dma_engines[(c // 2) % 2].dma_start(dst, src)
            for i, (dst, src) in enumerate(pending_outs):
                dma_engines[i % 2].dma_start(dst, src)
            pending_outs = []
            # bank b = g*NJB + jb
            ps = ps_t

            def _mm(c, g, jb, ps=ps, x_sb=x_sb):
                nc.tensor.matmul(
                    ps[g * NJB + jb][:],
                    lhsT=wt_sb[:, c, g, :],
                    rhs=x_sb[c][:, jb * JBLK : (jb + 1) * JBLK],
                    start=(c == 0),
                    stop=(c == KD - 1),
                    skip_group_check=True,
                )

            ot = [ot_t[(sg % 2) * 2 + g] for g in range(2)]
            # ev tile e covers banks 2e, 2e+1 so Ln can run 1024 wide
            evs = ev_t

            def _exp(b, evs=evs, ps=ps):
                g = b // NJB
                nc.scalar.activation(
                    evs[b // 2][:, b % 2, :], ps[b][:], EXP, bias=bv_sb[:, g : g + 1]
                )

            def _ln(e, evs=evs, ot=ot):
                # ev tile e -> banks 2e,2e+1 -> half g = (2e)//NJB, cols
                g, lo = divmod(2 * e, NJB)
                nc.scalar.activation(
                    ot[g][:, lo * JBLK : (lo + 2) * JBLK], evs[e][:], LN, bias=1.0
                )

            if sg < NSG - 1:
                # streaming: PE trails the x DMA chunk by chunk; banks all
                # complete in the final c sweep, Exps chase it bank by bank.
                # The g1 banks enter ~2.6us late (stagger, sg>=1 only: sg0's
                # chunk 1 hasn't streamed in yet) so the previous
                # supergroup's Exp chain has freed them by then.
                if sg == 0:
                    order = [(c, g) for c in range(KD) for g in range(2)]
                else:
                    order = [(0, 0), (1, 0), (0, 1), (1, 1)] + [
                        (c, g) for c in range(2, KD) for g in range(2)
                    ]
                for c, g in order:
                    for jb in range(NJB):
                        _mm(c, g, jb)
                for b in range(2 * NJB):
                    _exp(b)
                for e in range(NJB):
                    _ln(e)
                # 1024-col pieces: 2KB DRAM runs, half the trigger count
                for g in range(2):
                    for pi in range(2):
                        w = SGC // 2
                        pending_outs.append(
                            (
                                out[g, :, col0 + pi * w : col0 + (pi + 1) * w],
                                ot[g][:, pi * w : (pi + 1) * w],
                            )
                        )
            else:
                # drain tail: bank-major (x already resident), epilogue and
                # out-DMA ride along per bank pair
                for b in range(2 * NJB):
                    g, jb = divmod(b, NJB)
                    for c in range(KD):
                        _mm(c, g, jb)
                    _exp(b)
                    if b % 2 == 1:
                        e = b // 2
                        _ln(e)
                        g2, lo = divmod(2 * e, NJB)
                        for pi in (lo, lo + 1):
                            dma_engines[pi % 2].dma_start(
                                out[g2, :, col0 + pi * JBLK : col0 + (pi + 1) * JBLK],
                                ot[g2][:, pi * JBLK : (pi + 1) * JBLK],
                            )
    nc.compile()
    return nc


def _get_nc():
    key = MM_DT_NAME
    if key not in _CACHE:
        _CACHE[key] = _build_bass()
    return _CACHE[key]


def _host_prep(x, Wd, bd, Wo, bo):
    np_dt = _np_dt()
    # device weights: rows 0:16 = -Wd (diag), 16:256 = Wo (off-diag, row-major)
    w_dev = np.empty((NOUT, D), np.float32)
    b_dev = np.empty(NOUT, np.float32)
    w_dev[:K] = -Wd
    b_dev[:K] = -bd
    w_dev[K:] = Wo
    b_dev[K:] = bo
    wtT = w_dev.T  # (D, 256)
    wt_blob = np.ascontiguousarray(
        wtT.reshape(KD, 128, 2, 128).transpose(1, 0, 2, 3)
    ).astype(np_dt)
    bvec = np.ascontiguousarray(b_dev.reshape(2, 128).T).astype(np.float32)
    in_maps = []
    for c in range(NCORES):
        xs = x[c * BC : (c + 1) * BC]  # (BC, D)
        # -> (KD, 128, BC): element (c, p, t) = xs[t, c*128+p]
        xT = np.ascontiguousarray(xs.reshape(BC, KD, 128).transpose(1, 2, 0)).astype(
            np_dt
        )
        in_maps.append({"xT": xT, "wt": wt_blob, "bvec": bvec})
    return in_maps


def _install_env_shims():
    """The agent image's `antenv` stub lacks `axon_hooks`; bass_utils imports
    it on any trace=True/BASS_TRACE run. Provide it (wired to the ctypes NTFF
    hook when available), and skip the S3 artifact upload (no egress)."""
    if "antenv.axon_hooks" in sys.modules:
        return
    import types

    try:
        import antenv
    except ImportError:
        return
    if hasattr(antenv, "axon_hooks"):
        return
    mod = types.ModuleType("antenv.axon_hooks")
    hook = [None]
    try:
        from trn_agent_boot.trn_boot import _ntff_profile_via_ctypes

        hook[0] = _ntff_profile_via_ctypes("/opt/axon/libaxon_pjrt.so")
    except Exception:
        pass
    mod.set_axon_ntff_profile_hook = lambda h: hook.__setitem__(0, h)
    mod.get_axon_ntff_profile_hook = lambda: hook[0]
    sys.modules["antenv.axon_hooks"] = mod
    antenv.axon_hooks = mod

    import concourse.bass_utils as bu

    bu.upload_artifacts = lambda tmpdir: tmpdir


def _run(in_maps, **kwargs):
    _install_env_shims()
    nc = _get_nc()
    return run_bass_kernel_spmd(nc, in_maps, list(range(NCORES)), **kwargs)


def kernel(x, Wd, bd, Wo, bo, _bench_results=None, **kwargs):
    x = np.asarray(x, np.float32)
    in_maps = _host_prep(
        x,
        np.asarray(Wd, np.float32),
        np.asarray(bd, np.float32),
        np.asarray(Wo, np.float32),
        np.asarray(bo, np.float32),
    )
    res = _run(in_maps, **kwargs)
    if _bench_results is not None:
        _bench_results.append(res)
    perm = _perm()
    out_full = np.empty((B, NOUT), np.float32)
    for c in range(NCORES):
        v = np.asarray(res.results[c]["out"]).reshape(NOUT, BC).astype(np.float32)
        tmp = np.empty((NOUT, BC), np.float32)
        tmp[perm[:K]] = np.exp(-v[:K])  # diag: sigmoid(z_d)
        tmp[perm[K:]] = -v[K:]  # off-diag: -softplus
        out_full[c * BC : (c + 1) * BC] = tmp.T
    return out_full.reshape(B, K, K)


# revision 36
# speedup vs baseline: 1.1245x; 1.1245x over previous
"""Trainium2 Bass kernel for nn_CapacitanceMatrix.

C[b, i, j] = sigmoid(x[b]·Wd[i] + bd[i])        if i == j
           = -softplus(x[b]·Wo[m] + bo[m])      if i != j  (m = row-major off-diag idx)

Output-stationary layout: the 256 output rows (permuted: 16 diag rows first)
sit on PSUM partitions in 2 halves of 128; batch streams as the moving
operand in 512-col blocks, 8 PSUM banks <- (half g, block jb), accumulating
the 8 D-chunks. Every matmul pays its own stationary load (measured: no
elision on TRN2), so instruction order is chosen purely for pipelining:

- supergroups 0..2 run chunk-outer (c, g, jb) so the PE trails the streaming
  x DMA chunk by chunk;
- the last supergroup runs bank-major (g, jb, c) — its x is fully prefetched
  by then — so banks complete evenly through the group and the Exp/Ln/DMA
  drain tail shrinks from ~13us to ~4us.

Device computes v = softplus(W'x + b') uniformly for all rows, with diag rows
hosting W' = -Wd, b' = -bd so that v_diag = softplus(-z_d) = -ln(sigmoid(z_d)).
Host finishes: off-diag C = -v, diag C = exp(-v) = sigmoid(z_d). The bias
rides the Exp activation's per-partition bias AP (no PSUM seeding, no vector
engine). Output is fp16 (host upcasts), halving out-traffic: 21.5MB/core.
"""

import os
import sys

sys.path.insert(0, "/opt/trn_rl_repo")

from contextlib import ExitStack

import numpy as np

import concourse.bass as bass  # noqa: F401  (AP helpers)
import concourse.tile as tile
from concourse import bacc, mybir
from concourse.bass_utils import run_bass_kernel_spmd

B = 65536
D = 1024
K = 16
NOUT = K * K  # 256
NCORES = 8
BC = B // NCORES  # 8192 batch rows per core
KD = D // 128  # 8 contraction chunks
SGC = 2048  # supergroup width
NSG = BC // SGC  # 4
JBLK = 512  # cols per matmul / psum bank
NJB = SGC // JBLK  # 4

# matmul dtype for x / weights ("bfloat16" default)
MM_DT_NAME = os.environ.get("CAP_MM_DT", "bfloat16")

# uint8 fixed-point output encoding: softplus values live in [0, ~5.2]
# (z+b maxes near 2.6 off-diag / 5.1 diag for N(0,1) activations); quantum
# VMAX/255 = 0.024 is ~2e-2*scale/2, well inside the tolerance. Halves the
# out-DMA bytes vs fp16 (total HBM 21.5 -> 19.4 MB/core).
VMAX = 6.0
SCL = 255.0 / VMAX

_CACHE = {}

_ACT_TABLES_PATCHED = False


def _pin_act_table_set():
    """Force Exp and Ln to resolve to the single LUT set that holds both
    (`natural_log_exp_and_others`) so the Exp/Ln alternation doesn't thrash
    ACT_TABLE_LOADs."""
    global _ACT_TABLES_PATCHED
    if _ACT_TABLES_PATCHED:
        return
    import concourse.hw_specs as hw_specs

    orig = hw_specs.get_activation_tables

    def patched(arch):
        tables = {k: set(v) for k, v in orig(arch).items()}
        keep = "natural_log_exp_and_others"
        if keep in tables:
            for k, v in tables.items():
                if k != keep:
                    v.discard(mybir.ActivationFunctionType.Exp)
                    v.discard(mybir.ActivationFunctionType.Ln)
        return tables

    bacc.get_activation_tables = patched
    _ACT_TABLES_PATCHED = True


def _mm_dt():
    return getattr(mybir.dt, MM_DT_NAME)


def _np_dt():
    return mybir.dt.np(_mm_dt())


def _perm():
    """Device row r -> original flat output index (i*16+j)."""
    off_i, off_j = np.nonzero(~np.eye(K, dtype=bool))
    perm = np.empty(NOUT, np.int64)
    perm[:K] = np.arange(K) * (K + 1)
    perm[K:] = off_i * K + off_j
    return perm


def _build_bass():
    _pin_act_table_set()
    mm_dt = _mm_dt()
    f32 = mybir.dt.float32
    f16 = mybir.dt.float16
    nc = bacc.Bacc("TRN2", target_bir_lowering=False, debug=False)
    # x pre-tiled on host: [chunk, partition(d), col(batch)] per core
    xT = nc.dram_tensor("xT", [KD, 128, BC], mm_dt, kind="ExternalInput").ap()
    # wt[p, c, g, n] = W'^T[c*128+p, g*128+n]
    wt = nc.dram_tensor("wt", [128, KD, 2, 128], mm_dt, kind="ExternalInput").ap()
    # bvec[p, g] = b'[g*128+p]
    bvec = nc.dram_tensor("bvec", [128, 2], f32, kind="ExternalInput").ap()
    u8 = mybir.dt.uint8
    # out[g, r, col] = round(softplus * SCL + 0.5) for device row g*128+r
    out = nc.dram_tensor("out", [2, 128, BC], u8, kind="ExternalOutput").ap()

    EXP = mybir.ActivationFunctionType.Exp
    LN = mybir.ActivationFunctionType.Ln

    with tile.TileContext(nc) as tc, ExitStack() as ctx:
        const_pool = ctx.enter_context(tc.tile_pool(name="const", bufs=1))
        x_pool = ctx.enter_context(tc.tile_pool(name="x", bufs=3 * KD))
        ev_pool = ctx.enter_context(tc.tile_pool(name="ev", bufs=6))
        ot_pool = ctx.enter_context(tc.tile_pool(name="ot", bufs=8))
        psum_pool = ctx.enter_context(tc.tile_pool(name="ps", bufs=8, space="PSUM"))

        wt_sb = const_pool.tile([128, KD, 2, 128], mm_dt)
        bv_sb = const_pool.tile([128, 2], f32)
        # chunk-0 weights ride sync so the very first trigger serves the
        # first matmul; the rest trickle in on scalar (needed much later)
        nc.sync.dma_start(wt_sb[:, 0], wt[:, 0])
        nc.scalar.dma_start(bv_sb[:], bvec)
        for c in range(1, KD):
            nc.scalar.dma_start(wt_sb[:, c], wt[:, c])

        dma_engines = [nc.sync, nc.gpsimd]
        # All tiles are allocated once and rotated manually: the end-of-kernel
        # barrier pays ~115ns per logical tile per engine (~57 EVENT_SEMAPHORE
        # slices/engine with per-supergroup allocation), so fewer logical
        # tiles directly shrink the ~10us drain storm.
        # 2 supergroups of x in flight: with 3, sg2's prefetch DMAs compete
        # with sg0's just-in-time chunks for HBM and starve the PE early on
        NXT = 2 * KD
        x_t = [x_pool.tile([128, SGC], mm_dt, name="x") for _ in range(NXT)]
        ps_t = [psum_pool.tile([128, JBLK], f32, name="ps") for _ in range(2 * NJB)]
        ev_t = [ev_pool.tile([128, 2, JBLK], f32, name="ev") for _ in range(NJB)]
        ot_t = [ot_pool.tile([128, SGC], f16, name="ot") for _ in range(4)]
        oq_t = [ot_pool.tile([128, SGC], u8, name="oq") for _ in range(4)]

        pending_outs = []  # out DMAs deferred one supergroup (waits satisfied)
        for sg in range(NSG):
            col0 = sg * SGC
            x_sb = []
            for c in range(KD):
                xc = x_t[(sg * KD + c) % NXT]
                # chunk 0 of sg 0 lands fine-grained so the first matmul can
                # start within ~2us; everything else uses 2KB-run halves.
                # Pieces alternate sync/gpsimd so one sequencer's issue rate
                # never serializes a chunk's arrival.
                npc = 4 if (sg == 0 and c == 0) else 2
                w = SGC // npc
                for q in range(npc):
                    # sg0's last chunks ride scalar (idle until the first
                    # Exp at ~17us) so the 600ns-per-trigger issue rate on
                    # sync/gpsimd doesn't throttle early queue ramp-up
                    if sg == 0 and c >= 6:
                        eng = nc.scalar
                    else:
                        eng = dma_engines[(c * npc + q) % 2]
                    eng.dma_start(
                        xc[:, q * w : (q + 1) * w],
                        xT[c, :, col0 + q * w : col0 + (q + 1) * w],
                    )
                x_sb.append(xc)
                # trickle out one deferred out-DMA per odd chunk: spreads the
                # 1MB of out traffic through the supergroup instead of
                # bunching it against the first chunks' arrival
                if c % 2 == 1 and pending_outs:
                    dst, src = pending_outs.pop(0)
                    dma_engines[(c // 2) % 2].dma_start(dst, src)
            for i, (dst, src) in enumerate(pending_outs):
                dma_engines[i % 2].dma_start(dst, src)
            pending_outs = []
            # bank b = g*NJB + jb
            ps = ps_t

            def _mm(c, g, jb, ps=ps, x_sb=x_sb):
                nc.tensor.matmul(
                    ps[g * NJB + jb][:],
                    lhsT=wt_sb[:, c, g, :],
                    rhs=x_sb[c][:, jb * JBLK : (jb + 1) * JBLK],
                    start=(c == 0),
                    stop=(c == KD - 1),
                    skip_group_check=True,
                )

            ot = [ot_t[(sg % 2) * 2 + g] for g in range(2)]
            oq = [oq_t[(sg % 2) * 2 + g] for g in range(2)]
            # ev tile e covers banks 2e, 2e+1 so Ln can run 1024 wide
            evs = ev_t

            def _exp(b, evs=evs, ps=ps):
                g = b // NJB
                nc.scalar.activation(
                    evs[b // 2][:, b % 2, :], ps[b][:], EXP, bias=bv_sb[:, g : g + 1]
                )

            def _ln(e, evs=evs, ot=ot):
                # ev tile e -> banks 2e,2e+1 -> half g = (2e)//NJB, cols
                g, lo = divmod(2 * e, NJB)
                nc.scalar.activation(
                    ot[g][:, lo * JBLK : (lo + 2) * JBLK], evs[e][:], LN, bias=1.0
                )

            if sg < NSG - 1:
                # streaming: PE trails the x DMA chunk by chunk; banks all
                # complete in the final c sweep, Exps chase it bank by bank.
                # The g1 banks enter ~2.6us late (stagger, sg>=1 only: sg0's
                # chunk 1 hasn't streamed in yet) so the previous
                # supergroup's Exp chain has freed them by then.
                if sg == 0:
                    order = [(c, g) for c in range(KD) for g in range(2)]
                else:
                    order = [(0, 0), (1, 0), (0, 1), (1, 1)] + [
                        (c, g) for c in range(2, KD) for g in range(2)
                    ]
                for c, g in order:
                    for jb in range(NJB):
                        _mm(c, g, jb)
                for b in range(2 * NJB):
                    _exp(b)
                for e in range(NJB):
                    _ln(e)
                # quantize on the (idle) vector engine: q = v*SCL + 0.5, the
                # +0.5 keeps LUT jitter near v=0 from going negative in the
                # uint8 cast; host decodes with the matching offset
                for g in range(2):
                    nc.vector.tensor_scalar(
                        oq[g][:],
                        ot[g][:],
                        SCL,
                        0.5,
                        mybir.AluOpType.mult,
                        mybir.AluOpType.add,
                    )
                # full-supergroup u8 pieces: 2KB DRAM runs, 2 triggers/sg
                for g in range(2):
                    pending_outs.append(
                        (out[g, :, col0 : col0 + SGC], oq[g][:])
                    )
            else:
                # drain tail: bank-major (x already resident), epilogue and
                # out-DMA ride along per bank pair
                for b in range(2 * NJB):
                    g, jb = divmod(b, NJB)
                    for c in range(KD):
                        _mm(c, g, jb)
                    _exp(b)
                    if b % 2 == 1:
                        e = b // 2
                        _ln(e)
                        g2, lo = divmod(2 * e, NJB)
                        sl = slice(lo * JBLK, (lo + 2) * JBLK)
                        nc.vector.tensor_scalar(
                            oq[g2][:, sl],
                            ot[g2][:, sl],
                            SCL,
                            0.5,
                            mybir.AluOpType.mult,
                            mybir.AluOpType.add,
                        )
                        dma_engines[e % 2].dma_start(
                            out[g2, :, col0 + lo * JBLK : col0 + (lo + 2) * JBLK],
                            oq[g2][:, sl],
                        )
    nc.compile()
    return nc


def _get_nc():
    key = MM_DT_NAME
    if key not in _CACHE:
        _CACHE[key] = _build_bass()
    return _CACHE[key]


def _host_prep(x, Wd, bd, Wo, bo):
    np_dt = _np_dt()
    # device weights: rows 0:16 = -Wd (diag), 16:256 = Wo (off-diag, row-major)
    w_dev = np.empty((NOUT, D), np.float32)
    b_dev = np.empty(NOUT, np.float32)
    w_dev[:K] = -Wd
    b_dev[:K] = -bd
    w_dev[K:] = Wo
    b_dev[K:] = bo
    wtT = w_dev.T  # (D, 256)
    wt_blob = np.ascontiguousarray(
        wtT.reshape(KD, 128, 2, 128).transpose(1, 0, 2, 3)
    ).astype(np_dt)
    bvec = np.ascontiguousarray(b_dev.reshape(2, 128).T).astype(np.float32)
    in_maps = []
    for c in range(NCORES):
        xs = x[c * BC : (c + 1) * BC]  # (BC, D)
        # -> (KD, 128, BC): element (c, p, t) = xs[t, c*128+p]
        xT = np.ascontiguousarray(xs.reshape(BC, KD, 128).transpose(1, 2, 0)).astype(
            np_dt
        )
        in_maps.append({"xT": xT, "wt": wt_blob, "bvec": bvec})
    return in_maps


def _install_env_shims():
    """The agent image's `antenv` stub lacks `axon_hooks`; bass_utils imports
    it on any trace=True/BASS_TRACE run. Provide it (wired to the ctypes NTFF
    hook when available), and skip the S3 artifact upload (no egress)."""
    if "antenv.axon_hooks" in sys.modules:
        return
    import types

    try:
        import antenv
    except ImportError:
        return
    if hasattr(antenv, "axon_hooks"):
        return
    mod = types.ModuleType("antenv.axon_hooks")
    hook = [None]
    try:
        from trn_agent_boot.trn_boot import _ntff_profile_via_ctypes

        hook[0] = _ntff_profile_via_ctypes("/opt/axon/libaxon_pjrt.so")
    except Exception:
        pass
    mod.set_axon_ntff_profile_hook = lambda h: hook.__setitem__(0, h)
    mod.get_axon_ntff_profile_hook = lambda: hook[0]
    sys.modules["antenv.axon_hooks"] = mod
    antenv.axon_hooks = mod

    import concourse.bass_utils as bu

    bu.upload_artifacts = lambda tmpdir: tmpdir


def _run(in_maps, **kwargs):
    _install_env_shims()
    nc = _get_nc()
    return run_bass_kernel_spmd(nc, in_maps, list(range(NCORES)), **kwargs)


def kernel(x, Wd, bd, Wo, bo, _bench_results=None, **kwargs):
    x = np.asarray(x, np.float32)
    in_maps = _host_prep(
        x,
        np.asarray(Wd, np.float32),
        np.asarray(bd, np.float32),
        np.asarray(Wo, np.float32),
        np.asarray(bo, np.float32),
    )
    res = _run(in_maps, **kwargs)
    if _bench_results is not None:
        _bench_results.append(res)
    perm = _perm()
    out_full = np.empty((B, NOUT), np.float32)
    for c in range(NCORES):
        q = np.asarray(res.results[c]["out"]).reshape(NOUT, BC).astype(np.float32)
        # device stored q = cast(v*SCL + 0.5); decode the midpoint
        v = (q - 0.5) * (1.0 / SCL)
        np.maximum(v, 0.0, out=v)
        tmp = np.empty((NOUT, BC), np.float32)
        tmp[perm[:K]] = np.exp(-v[:K])  # diag: sigmoid(z_d)
        tmp[perm[K:]] = -v[K:]  # off-diag: -softplus
        out_full[c * BC : (c + 1) * BC] = tmp.T
    return out_full.reshape(B, K, K)


# revision 37
# speedup vs baseline: 1.1408x; 1.0145x over previous
"""Trainium2 Bass kernel for nn_CapacitanceMatrix.

C[b, i, j] = sigmoid(x[b]·Wd[i] + bd[i])        if i == j
           = -softplus(x[b]·Wo[m] + bo[m])      if i != j  (m = row-major off-diag idx)

Output-stationary layout: the 256 output rows (permuted: 16 diag rows first)
sit on PSUM partitions in 2 halves of 128; batch streams as the moving
operand in 512-col blocks, 8 PSUM banks <- (half g, block jb), accumulating
the 8 D-chunks. Every matmul pays its own stationary load (measured: no
elision on TRN2), so instruction order is chosen purely for pipelining:

- supergroups 0..2 run chunk-outer (c, g, jb) so the PE trails the streaming
  x DMA chunk by chunk;
- the last supergroup runs bank-major (g, jb, c) — its x is fully prefetched
  by then — so banks complete evenly through the group and the Exp/Ln/DMA
  drain tail shrinks from ~13us to ~4us.

Device computes v = softplus(W'x + b') uniformly for all rows, with diag rows
hosting W' = -Wd, b' = -bd so that v_diag = softplus(-z_d) = -ln(sigmoid(z_d)).
Host finishes: off-diag C = -v, diag C = exp(-v) = sigmoid(z_d). The bias
rides the Exp activation's per-partition bias AP (no PSUM seeding, no vector
engine). Output is fp16 (host upcasts), halving out-traffic: 21.5MB/core.
"""

import os
import sys

sys.path.insert(0, "/opt/trn_rl_repo")

from contextlib import ExitStack

import numpy as np

import concourse.bass as bass  # noqa: F401  (AP helpers)
import concourse.tile as tile
from concourse import bacc, mybir
from concourse.bass_utils import run_bass_kernel_spmd

B = 65536
D = 1024
K = 16
NOUT = K * K  # 256
NCORES = 8
BC = B // NCORES  # 8192 batch rows per core
KD = D // 128  # 8 contraction chunks
SGC = 2048  # supergroup width
NSG = BC // SGC  # 4
JBLK = 512  # cols per matmul / psum bank
NJB = SGC // JBLK  # 4

# matmul dtype for x / weights ("bfloat16" default)
MM_DT_NAME = os.environ.get("CAP_MM_DT", "bfloat16")

_CACHE = {}

_ACT_TABLES_PATCHED = False


def _pin_act_table_set():
    """Force Exp and Ln to resolve to the single LUT set that holds both
    (`natural_log_exp_and_others`) so the Exp/Ln alternation doesn't thrash
    ACT_TABLE_LOADs."""
    global _ACT_TABLES_PATCHED
    if _ACT_TABLES_PATCHED:
        return
    import concourse.hw_specs as hw_specs

    orig = hw_specs.get_activation_tables

    def patched(arch):
        tables = {k: set(v) for k, v in orig(arch).items()}
        keep = "natural_log_exp_and_others"
        if keep in tables:
            for k, v in tables.items():
                if k != keep:
                    v.discard(mybir.ActivationFunctionType.Exp)
                    v.discard(mybir.ActivationFunctionType.Ln)
        return tables

    bacc.get_activation_tables = patched
    _ACT_TABLES_PATCHED = True


def _mm_dt():
    return getattr(mybir.dt, MM_DT_NAME)


def _np_dt():
    return mybir.dt.np(_mm_dt())


def _perm():
    """Device row r -> original flat output index (i*16+j)."""
    off_i, off_j = np.nonzero(~np.eye(K, dtype=bool))
    perm = np.empty(NOUT, np.int64)
    perm[:K] = np.arange(K) * (K + 1)
    perm[K:] = off_i * K + off_j
    return perm


def _build_bass():
    _pin_act_table_set()
    mm_dt = _mm_dt()
    f32 = mybir.dt.float32
    f16 = mybir.dt.float16
    nc = bacc.Bacc("TRN2", target_bir_lowering=False, debug=False)
    # x pre-tiled on host: [chunk, partition(d), col(batch)] per core
    xT = nc.dram_tensor("xT", [KD, 128, BC], mm_dt, kind="ExternalInput").ap()
    # wt[p, c, g, n] = W'^T[c*128+p, g*128+n]
    wt = nc.dram_tensor("wt", [128, KD, 2, 128], mm_dt, kind="ExternalInput").ap()
    # bvec[p, g] = b'[g*128+p]
    bvec = nc.dram_tensor("bvec", [128, 2], f32, kind="ExternalInput").ap()
    # out[g, r, col] = softplus value for device row g*128+r, batch col
    out = nc.dram_tensor("out", [2, 128, BC], f16, kind="ExternalOutput").ap()

    EXP = mybir.ActivationFunctionType.Exp
    LN = mybir.ActivationFunctionType.Ln

    with tile.TileContext(nc) as tc, ExitStack() as ctx:
        const_pool = ctx.enter_context(tc.tile_pool(name="const", bufs=1))
        x_pool = ctx.enter_context(tc.tile_pool(name="x", bufs=3 * KD))
        ev_pool = ctx.enter_context(tc.tile_pool(name="ev", bufs=6))
        ot_pool = ctx.enter_context(tc.tile_pool(name="ot", bufs=4))
        psum_pool = ctx.enter_context(tc.tile_pool(name="ps", bufs=8, space="PSUM"))

        wt_sb = const_pool.tile([128, KD, 2, 128], mm_dt)
        bv_sb = const_pool.tile([128, 2], f32)
        # chunk-0 weights ride sync so the very first trigger serves the
        # first matmul; the rest trickle in on scalar (needed much later)
        nc.sync.dma_start(wt_sb[:, 0], wt[:, 0])
        nc.scalar.dma_start(bv_sb[:], bvec)
        for c in range(1, KD):
            nc.scalar.dma_start(wt_sb[:, c], wt[:, c])

        dma_engines = [nc.sync, nc.gpsimd]
        # All tiles are allocated once and rotated manually: the end-of-kernel
        # barrier pays ~115ns per logical tile per engine (~57 EVENT_SEMAPHORE
        # slices/engine with per-supergroup allocation), so fewer logical
        # tiles directly shrink the ~10us drain storm.
        # 2 supergroups of x in flight: with 3, sg2's prefetch DMAs compete
        # with sg0's just-in-time chunks for HBM and starve the PE early on
        NXT = 2 * KD
        x_t = [x_pool.tile([128, SGC], mm_dt, name="x") for _ in range(NXT)]
        ps_t = [psum_pool.tile([128, JBLK], f32, name="ps") for _ in range(2 * NJB)]
        ev_t = [ev_pool.tile([128, 2, JBLK], f32, name="ev") for _ in range(NJB)]
        ot_t = [ot_pool.tile([128, SGC], f16, name="ot") for _ in range(4)]

        pending_outs = []  # out DMAs deferred one supergroup (waits satisfied)
        for sg in range(NSG):
            col0 = sg * SGC
            x_sb = []
            for c in range(KD):
                xc = x_t[(sg * KD + c) % NXT]
                # chunk 0 of sg 0 lands fine-grained so the first matmul can
                # start within ~2us; everything else uses 2KB-run halves.
                # Pieces alternate sync/gpsimd so one sequencer's issue rate
                # never serializes a chunk's arrival.
                npc = 4 if (sg == 0 and c == 0) else 2
                w = SGC // npc
                for q in range(npc):
                    # sg0's last chunks ride scalar (idle until the first
                    # Exp at ~17us) so the 600ns-per-trigger issue rate on
                    # sync/gpsimd doesn't throttle early queue ramp-up
                    if sg == 0 and c >= 6:
                        eng = nc.scalar
                    else:
                        eng = dma_engines[(c * npc + q) % 2]
                    eng.dma_start(
                        xc[:, q * w : (q + 1) * w],
                        xT[c, :, col0 + q * w : col0 + (q + 1) * w],
                    )
                x_sb.append(xc)
                # trickle out one deferred out-DMA per odd chunk: spreads the
                # 1MB of out traffic through the supergroup instead of
                # bunching it against the first chunks' arrival
                if c % 2 == 1 and pending_outs:
                    dst, src = pending_outs.pop(0)
                    dma_engines[(c // 2) % 2].dma_start(dst, src)
            for i, (dst, src) in enumerate(pending_outs):
                dma_engines[i % 2].dma_start(dst, src)
            pending_outs = []
            # bank b = g*NJB + jb
            ps = ps_t

            def _mm(c, g, jb, ps=ps, x_sb=x_sb):
                nc.tensor.matmul(
                    ps[g * NJB + jb][:],
                    lhsT=wt_sb[:, c, g, :],
                    rhs=x_sb[c][:, jb * JBLK : (jb + 1) * JBLK],
                    start=(c == 0),
                    stop=(c == KD - 1),
                    skip_group_check=True,
                )

            ot = [ot_t[(sg % 2) * 2 + g] for g in range(2)]
            # ev tile e covers banks 2e, 2e+1 so Ln can run 1024 wide
            evs = ev_t

            def _exp(b, evs=evs, ps=ps):
                g = b // NJB
                nc.scalar.activation(
                    evs[b // 2][:, b % 2, :], ps[b][:], EXP, bias=bv_sb[:, g : g + 1]
                )

            def _ln(e, evs=evs, ot=ot):
                # ev tile e -> banks 2e,2e+1 -> half g = (2e)//NJB, cols
                g, lo = divmod(2 * e, NJB)
                nc.scalar.activation(
                    ot[g][:, lo * JBLK : (lo + 2) * JBLK], evs[e][:], LN, bias=1.0
                )

            if sg < NSG - 1:
                # streaming: PE trails the x DMA chunk by chunk; banks all
                # complete in the final c sweep, Exps chase it bank by bank.
                # The g1 banks enter ~2.6us late (stagger, sg>=1 only: sg0's
                # chunk 1 hasn't streamed in yet) so the previous
                # supergroup's Exp chain has freed them by then.
                if sg == 0:
                    order = [(c, g) for c in range(KD) for g in range(2)]
                else:
                    order = [(0, 0), (1, 0), (0, 1), (1, 1)] + [
                        (c, g) for c in range(2, KD) for g in range(2)
                    ]
                for c, g in order:
                    for jb in range(NJB):
                        _mm(c, g, jb)
                for b in range(2 * NJB):
                    _exp(b)
                for e in range(NJB):
                    _ln(e)
                # 1024-col pieces: 2KB DRAM runs, half the trigger count
                for g in range(2):
                    for pi in range(2):
                        w = SGC // 2
                        pending_outs.append(
                            (
                                out[g, :, col0 + pi * w : col0 + (pi + 1) * w],
                                ot[g][:, pi * w : (pi + 1) * w],
                            )
                        )
            else:
                # drain tail: bank-major (x already resident), epilogue and
                # out-DMA ride along per bank pair
                for b in range(2 * NJB):
                    g, jb = divmod(b, NJB)
                    for c in range(KD):
                        _mm(c, g, jb)
                    _exp(b)
                    if b % 2 == 1:
                        e = b // 2
                        _ln(e)
                        g2, lo = divmod(2 * e, NJB)
                        for pi in (lo, lo + 1):
                            dma_engines[pi % 2].dma_start(
                                out[g2, :, col0 + pi * JBLK : col0 + (pi + 1) * JBLK],
                                ot[g2][:, pi * JBLK : (pi + 1) * JBLK],
                            )
    nc.compile()
    return nc


def _get_nc():
    key = MM_DT_NAME
    if key not in _CACHE:
        _CACHE[key] = _build_bass()
    return _CACHE[key]


def _host_prep(x, Wd, bd, Wo, bo):
    np_dt = _np_dt()
    # device weights: rows 0:16 = -Wd (diag), 16:256 = Wo (off-diag, row-major)
    w_dev = np.empty((NOUT, D), np.float32)
    b_dev = np.empty(NOUT, np.float32)
    w_dev[:K] = -Wd
    b_dev[:K] = -bd
    w_dev[K:] = Wo
    b_dev[K:] = bo
    wtT = w_dev.T  # (D, 256)
    wt_blob = np.ascontiguousarray(
        wtT.reshape(KD, 128, 2, 128).transpose(1, 0, 2, 3)
    ).astype(np_dt)
    bvec = np.ascontiguousarray(b_dev.reshape(2, 128).T).astype(np.float32)
    in_maps = []
    for c in range(NCORES):
        xs = x[c * BC : (c + 1) * BC]  # (BC, D)
        # -> (KD, 128, BC): element (c, p, t) = xs[t, c*128+p]
        xT = np.ascontiguousarray(xs.reshape(BC, KD, 128).transpose(1, 2, 0)).astype(
            np_dt
        )
        in_maps.append({"xT": xT, "wt": wt_blob, "bvec": bvec})
    return in_maps


def _install_env_shims():
    """The agent image's `antenv` stub lacks `axon_hooks`; bass_utils imports
    it on any trace=True/BASS_TRACE run. Provide it (wired to the ctypes NTFF
    hook when available), and skip the S3 artifact upload (no egress)."""
    if "antenv.axon_hooks" in sys.modules:
        return
    import types

    try:
        import antenv
    except ImportError:
        return
    if hasattr(antenv, "axon_hooks"):
        return
    mod = types.ModuleType("antenv.axon_hooks")
    hook = [None]
    try:
        from trn_agent_boot.trn_boot import _ntff_profile_via_ctypes

        hook[0] = _ntff_profile_via_ctypes("/opt/axon/libaxon_pjrt.so")
    except Exception:
        pass
    mod.set_axon_ntff_profile_hook = lambda h: hook.__setitem__(0, h)
    mod.get_axon_ntff_profile_hook = lambda: hook[0]
    sys.modules["antenv.axon_hooks"] = mod
    antenv.axon_hooks = mod

    import concourse.bass_utils as bu

    bu.upload_artifacts = lambda tmpdir: tmpdir


def _run(in_maps, **kwargs):
    _install_env_shims()
    nc = _get_nc()
    return run_bass_kernel_spmd(nc, in_maps, list(range(NCORES)), **kwargs)


def kernel(x, Wd, bd, Wo, bo, _bench_results=None, **kwargs):
    x = np.asarray(x, np.float32)
    in_maps = _host_prep(
        x,
        np.asarray(Wd, np.float32),
        np.asarray(bd, np.float32),
        np.asarray(Wo, np.float32),
        np.asarray(bo, np.float32),
    )
    res = _run(in_maps, **kwargs)
    if _bench_results is not None:
        _bench_results.append(res)
    perm = _perm()
    out_full = np.empty((B, NOUT), np.float32)
    for c in range(NCORES):
        v = np.asarray(res.results[c]["out"]).reshape(NOUT, BC).astype(np.float32)
        tmp = np.empty((NOUT, BC), np.float32)
        tmp[perm[:K]] = np.exp(-v[:K])  # diag: sigmoid(z_d)
        tmp[perm[K:]] = -v[K:]  # off-diag: -softplus
        out_full[c * BC : (c + 1) * BC] = tmp.T
    return out_full.reshape(B, K, K)


# revision 39
# speedup vs baseline: 1.1575x; 1.0146x over previous
"""Trainium2 Bass kernel for nn_CapacitanceMatrix.

C[b, i, j] = sigmoid(x[b]·Wd[i] + bd[i])        if i == j
           = -softplus(x[b]·Wo[m] + bo[m])      if i != j  (m = row-major off-diag idx)

Output-stationary layout: the 256 output rows (permuted: 16 diag rows first)
sit on PSUM partitions in 2 halves of 128; batch streams as the moving
operand in 512-col blocks, 8 PSUM banks <- (half g, block jb), accumulating
the 8 D-chunks. Every matmul pays its own stationary load (measured: no
elision on TRN2), so instruction order is chosen purely for pipelining:

- supergroups 0..2 run chunk-outer (c, g, jb) so the PE trails the streaming
  x DMA chunk by chunk;
- the last supergroup runs bank-major (g, jb, c) — its x is fully prefetched
  by then — so banks complete evenly through the group and the Exp/Ln/DMA
  drain tail shrinks from ~13us to ~4us.

Device computes v = softplus(W'x + b') uniformly for all rows, with diag rows
hosting W' = -Wd, b' = -bd so that v_diag = softplus(-z_d) = -ln(sigmoid(z_d)).
Host finishes: off-diag C = -v, diag C = exp(-v) = sigmoid(z_d). The bias
rides the Exp activation's per-partition bias AP (no PSUM seeding, no vector
engine). Output is fp16 (host upcasts), halving out-traffic: 21.5MB/core.
"""

import os
import sys

sys.path.insert(0, "/opt/trn_rl_repo")

from contextlib import ExitStack

import numpy as np

import concourse.bass as bass  # noqa: F401  (AP helpers)
import concourse.tile as tile
from concourse import bacc, mybir
from concourse.bass_utils import run_bass_kernel_spmd

B = 65536
D = 1024
K = 16
NOUT = K * K  # 256
NCORES = 8
BC = B // NCORES  # 8192 batch rows per core
KD = D // 128  # 8 contraction chunks
SGC = 2048  # supergroup width
NSG = BC // SGC  # 4
JBLK = 512  # cols per matmul / psum bank
NJB = SGC // JBLK  # 4

# matmul dtype for x / weights ("bfloat16" default)
MM_DT_NAME = os.environ.get("CAP_MM_DT", "bfloat16")

_CACHE = {}

_ACT_TABLES_PATCHED = False


def _pin_act_table_set():
    """Force Exp and Ln to resolve to the single LUT set that holds both
    (`natural_log_exp_and_others`) so the Exp/Ln alternation doesn't thrash
    ACT_TABLE_LOADs."""
    global _ACT_TABLES_PATCHED
    if _ACT_TABLES_PATCHED:
        return
    import concourse.hw_specs as hw_specs

    orig = hw_specs.get_activation_tables

    def patched(arch):
        tables = {k: set(v) for k, v in orig(arch).items()}
        keep = "natural_log_exp_and_others"
        if keep in tables:
            for k, v in tables.items():
                if k != keep:
                    v.discard(mybir.ActivationFunctionType.Exp)
                    v.discard(mybir.ActivationFunctionType.Ln)
        return tables

    bacc.get_activation_tables = patched
    _ACT_TABLES_PATCHED = True


def _mm_dt():
    return getattr(mybir.dt, MM_DT_NAME)


def _np_dt():
    return mybir.dt.np(_mm_dt())


def _perm():
    """Device row r -> original flat output index (i*16+j)."""
    off_i, off_j = np.nonzero(~np.eye(K, dtype=bool))
    perm = np.empty(NOUT, np.int64)
    perm[:K] = np.arange(K) * (K + 1)
    perm[K:] = off_i * K + off_j
    return perm


def _build_bass():
    _pin_act_table_set()
    mm_dt = _mm_dt()
    f32 = mybir.dt.float32
    f16 = mybir.dt.float16
    nc = bacc.Bacc("TRN2", target_bir_lowering=False, debug=False)
    # x pre-tiled on host: [chunk, partition(d), col(batch)] per core
    xT = nc.dram_tensor("xT", [KD, 128, BC], mm_dt, kind="ExternalInput").ap()
    # wt[p, c, g, n] = W'^T[c*128+p, g*128+n]
    wt = nc.dram_tensor("wt", [128, KD, 2, 128], mm_dt, kind="ExternalInput").ap()
    # bvec[p, g] = b'[g*128+p]
    bvec = nc.dram_tensor("bvec", [128, 2], f32, kind="ExternalInput").ap()
    # out[g, r, col] = softplus value for device row g*128+r, batch col
    out = nc.dram_tensor("out", [2, 128, BC], f16, kind="ExternalOutput").ap()

    EXP = mybir.ActivationFunctionType.Exp
    LN = mybir.ActivationFunctionType.Ln

    with tile.TileContext(nc) as tc, ExitStack() as ctx:
        const_pool = ctx.enter_context(tc.tile_pool(name="const", bufs=1))
        x_pool = ctx.enter_context(tc.tile_pool(name="x", bufs=3 * KD))
        ev_pool = ctx.enter_context(tc.tile_pool(name="ev", bufs=6))
        ot_pool = ctx.enter_context(tc.tile_pool(name="ot", bufs=4))
        psum_pool = ctx.enter_context(tc.tile_pool(name="ps", bufs=8, space="PSUM"))

        wt_sb = const_pool.tile([128, KD, 2, 128], mm_dt)
        bv_sb = const_pool.tile([128, 2], f32)
        # chunk-0 weights ride sync so the very first trigger serves the
        # first matmul; the rest trickle in on scalar (needed much later)
        nc.sync.dma_start(wt_sb[:, 0], wt[:, 0])
        nc.scalar.dma_start(bv_sb[:], bvec)
        for c in range(1, KD):
            nc.scalar.dma_start(wt_sb[:, c], wt[:, c])

        dma_engines = [nc.sync, nc.gpsimd]
        # All tiles are allocated once and rotated manually: the end-of-kernel
        # barrier pays ~115ns per logical tile per engine (~57 EVENT_SEMAPHORE
        # slices/engine with per-supergroup allocation), so fewer logical
        # tiles directly shrink the ~10us drain storm.
        # 2 supergroups of x in flight: with 3, sg2's prefetch DMAs compete
        # with sg0's just-in-time chunks for HBM and starve the PE early on
        NXT = 2 * KD
        x_t = [x_pool.tile([128, SGC], mm_dt, name="x") for _ in range(NXT)]
        ps_t = [psum_pool.tile([128, JBLK], f32, name="ps") for _ in range(2 * NJB)]
        ev_t = [ev_pool.tile([128, 2, JBLK], f32, name="ev") for _ in range(NJB)]
        ot_t = [ot_pool.tile([128, SGC], f16, name="ot") for _ in range(4)]

        pending_outs = []  # out DMAs deferred one supergroup (waits satisfied)
        for sg in range(NSG):
            col0 = sg * SGC
            x_sb = []
            for c in range(KD):
                xc = x_t[(sg * KD + c) % NXT]
                # chunk 0 of sg 0 lands as 256-col pieces: the first matmul
                # needs only pieces 0+1 (64KB each, parallel engines), so PE
                # starts ~9.4us instead of ~11.6. Everything else uses
                # 2KB-run halves. Pieces alternate sync/gpsimd so one
                # sequencer's issue rate never serializes a chunk's arrival.
                npc = 8 if (sg == 0 and c == 0) else 2
                w = SGC // npc
                for q in range(npc):
                    # sg0's last chunks ride scalar (idle until the first
                    # Exp at ~17us) so the 600ns-per-trigger issue rate on
                    # sync/gpsimd doesn't throttle early queue ramp-up
                    if sg == 0 and c >= 6:
                        eng = nc.scalar
                    else:
                        eng = dma_engines[(c * npc + q) % 2]
                    eng.dma_start(
                        xc[:, q * w : (q + 1) * w],
                        xT[c, :, col0 + q * w : col0 + (q + 1) * w],
                    )
                x_sb.append(xc)
                # trickle out one deferred out-DMA per odd chunk: spreads the
                # 1MB of out traffic through the supergroup instead of
                # bunching it against the first chunks' arrival
                if c % 2 == 1 and pending_outs:
                    dst, src = pending_outs.pop(0)
                    dma_engines[(c // 2) % 2].dma_start(dst, src)
            for i, (dst, src) in enumerate(pending_outs):
                dma_engines[i % 2].dma_start(dst, src)
            pending_outs = []
            # bank b = g*NJB + jb
            ps = ps_t

            def _mm(c, g, jb, ps=ps, x_sb=x_sb):
                nc.tensor.matmul(
                    ps[g * NJB + jb][:],
                    lhsT=wt_sb[:, c, g, :],
                    rhs=x_sb[c][:, jb * JBLK : (jb + 1) * JBLK],
                    start=(c == 0),
                    stop=(c == KD - 1),
                    skip_group_check=True,
                )

            ot = [ot_t[(sg % 2) * 2 + g] for g in range(2)]
            # ev tile e covers banks 2e, 2e+1 so Ln can run 1024 wide
            evs = ev_t

            def _exp(b, evs=evs, ps=ps):
                g = b // NJB
                nc.scalar.activation(
                    evs[b // 2][:, b % 2, :], ps[b][:], EXP, bias=bv_sb[:, g : g + 1]
                )

            def _ln(e, evs=evs, ot=ot):
                # ev tile e -> banks 2e,2e+1 -> half g = (2e)//NJB, cols
                g, lo = divmod(2 * e, NJB)
                nc.scalar.activation(
                    ot[g][:, lo * JBLK : (lo + 2) * JBLK], evs[e][:], LN, bias=1.0
                )

            if sg < NSG - 1:
                # streaming: PE trails the x DMA chunk by chunk; banks all
                # complete in the final c sweep, Exps chase it bank by bank.
                # The g1 banks enter ~2.6us late (stagger, sg>=1 only: sg0's
                # chunk 1 hasn't streamed in yet) so the previous
                # supergroup's Exp chain has freed them by then.
                if sg == 0:
                    order = [(c, g) for c in range(KD) for g in range(2)]
                else:
                    order = [(0, 0), (1, 0), (0, 1), (1, 1)] + [
                        (c, g) for c in range(2, KD) for g in range(2)
                    ]
                for c, g in order:
                    for jb in range(NJB):
                        _mm(c, g, jb)
                for b in range(2 * NJB):
                    _exp(b)
                for e in range(NJB):
                    _ln(e)
                # 1024-col pieces: 2KB DRAM runs, half the trigger count
                for g in range(2):
                    for pi in range(2):
                        w = SGC // 2
                        pending_outs.append(
                            (
                                out[g, :, col0 + pi * w : col0 + (pi + 1) * w],
                                ot[g][:, pi * w : (pi + 1) * w],
                            )
                        )
            else:
                # drain tail: bank-major (x already resident), epilogue and
                # out-DMA ride along per bank pair
                for b in range(2 * NJB):
                    g, jb = divmod(b, NJB)
                    for c in range(KD):
                        _mm(c, g, jb)
                    _exp(b)
                    if b % 2 == 1:
                        e = b // 2
                        _ln(e)
                        g2, lo = divmod(2 * e, NJB)
                        # final pair split 4 ways so the drain tail rides 4
                        # queues (~1.4us) instead of 2 (~2.7us)
                        npo = 4 if e == NJB - 1 else 2
                        w = 2 * JBLK // npo
                        for pi in range(npo):
                            c0_ = col0 + lo * JBLK + pi * w
                            dma_engines[pi % 2].dma_start(
                                out[g2, :, c0_ : c0_ + w],
                                ot[g2][:, lo * JBLK + pi * w : lo * JBLK + (pi + 1) * w],
                            )
    nc.compile()
    return nc


def _get_nc():
    key = MM_DT_NAME
    if key not in _CACHE:
        _CACHE[key] = _build_bass()
    return _CACHE[key]


def _host_prep(x, Wd, bd, Wo, bo):
    np_dt = _np_dt()
    # device weights: rows 0:16 = -Wd (diag), 16:256 = Wo (off-diag, row-major)
    w_dev = np.empty((NOUT, D), np.float32)
    b_dev = np.empty(NOUT, np.float32)
    w_dev[:K] = -Wd
    b_dev[:K] = -bd
    w_dev[K:] = Wo
    b_dev[K:] = bo
    wtT = w_dev.T  # (D, 256)
    wt_blob = np.ascontiguousarray(
        wtT.reshape(KD, 128, 2, 128).transpose(1, 0, 2, 3)
    ).astype(np_dt)
    bvec = np.ascontiguousarray(b_dev.reshape(2, 128).T).astype(np.float32)
    in_maps = []
    for c in range(NCORES):
        xs = x[c * BC : (c + 1) * BC]  # (BC, D)
        # -> (KD, 128, BC): element (c, p, t) = xs[t, c*128+p]
        xT = np.ascontiguousarray(xs.reshape(BC, KD, 128).transpose(1, 2, 0)).astype(
            np_dt
        )
        in_maps.append({"xT": xT, "wt": wt_blob, "bvec": bvec})
    return in_maps


def _install_env_shims():
    """The agent image's `antenv` stub lacks `axon_hooks`; bass_utils imports
    it on any trace=True/BASS_TRACE run. Provide it (wired to the ctypes NTFF
    hook when available), and skip the S3 artifact upload (no egress)."""
    if "antenv.axon_hooks" in sys.modules:
        return
    import types

    try:
        import antenv
    except ImportError:
        return
    if hasattr(antenv, "axon_hooks"):
        return
    mod = types.ModuleType("antenv.axon_hooks")
    hook = [None]
    try:
        from trn_agent_boot.trn_boot import _ntff_profile_via_ctypes

        hook[0] = _ntff_profile_via_ctypes("/opt/axon/libaxon_pjrt.so")
    except Exception:
        pass
    mod.set_axon_ntff_profile_hook = lambda h: hook.__setitem__(0, h)
    mod.get_axon_ntff_profile_hook = lambda: hook[0]
    sys.modules["antenv.axon_hooks"] = mod
    antenv.axon_hooks = mod

    import concourse.bass_utils as bu

    bu.upload_artifacts = lambda tmpdir: tmpdir


def _run(in_maps, **kwargs):
    _install_env_shims()
    nc = _get_nc()
    return run_bass_kernel_spmd(nc, in_maps, list(range(NCORES)), **kwargs)


def kernel(x, Wd, bd, Wo, bo, _bench_results=None, **kwargs):
    x = np.asarray(x, np.float32)
    in_maps = _host_prep(
        x,
        np.asarray(Wd, np.float32),
        np.asarray(bd, np.float32),
        np.asarray(Wo, np.float32),
        np.asarray(bo, np.float32),
    )
    res = _run(in_maps, **kwargs)
    if _bench_results is not None:
        _bench_results.append(res)
    perm = _perm()
    out_full = np.empty((B, NOUT), np.float32)
    for c in range(NCORES):
        v = np.asarray(res.results[c]["out"]).reshape(NOUT, BC).astype(np.float32)
        tmp = np.empty((NOUT, BC), np.float32)
        tmp[perm[:K]] = np.exp(-v[:K])  # diag: sigmoid(z_d)
        tmp[perm[K:]] = -v[K:]  # off-diag: -softplus
        out_full[c * BC : (c + 1) * BC] = tmp.T
    return out_full.reshape(B, K, K)
